# revision 7
# baseline (speedup 1.0000x reference)
"""CrossCosineEmbeddingLoss kernel for 8 trn2 NeuronCores (v6).

loss = mean over all (i,j) of: 1 - cos(x_i, y_j) if i==j else relu(cos(x_i, y_j))

Identity:  total = sum_ij relu(sim_ij) + sum_i (1 - sim_ii - relu(sim_ii))

Sharding (2x4 grid): core c = (bi, bj), bi = c // 2, bj = c % 2.
  x rows [2048*bi, 2048*(bi+1)) x y rows [4096*bj, 4096*(bj+1)).
Each core computes sum_ij relu(x_hat_i . y_j) / ||y_j|| over its block.
Diag correction only used from cores whose x block lies in their y range.

Per-core pipeline:
  - x: 2 half-DMAs; DVE STT sumsq per tile; rsqrt; DVE scale-cast to bf16;
    PE transpose; ACT copy -> x_hatT  (pipelined at half granularity)
  - y: SWDGE cast-DMA fp32->bf16 (2 chunks); PE transpose + DVE copy -> yT.
    Norms chunk 0: GpSimd squares (natural) + DVE segmented reduce (fast
    path for rny). Chunk 1 + diag: GpSimd squares of transposed tiles +
    N=1 ones-matmuls on PE (column sums) + one small DVE copy.
  - main: 32 j-tiles, each [128j, 2048i] fp32 PSUM (4 banks, pool bufs=2):
    4 bf16 matmuls (N=512); tile split between engines on distinct banks:
      ACT: activation(Relu, scale=rny_col, accum_out) on cols [0:1024)
      DVE: tensor_scalar(max 0, add, accum_out) on cols [1024:2048),
           R column post-scaled by rny afterwards
Host combines [128,2] partials; diag col used only from owner cores.
"""

import numpy as np

import concourse.bacc as bacc
import concourse.bass as bass
import concourse.tile as tile
from concourse import mybir
from concourse.bass_utils import run_bass_kernel_spmd
from concourse.masks import make_identity

N, D = 8192, 128
NCORES = 8
XI = 2048            # x rows per core
YJ = 4096            # y rows per core
TXI = XI // 128      # 16 x tiles
TYJ = YJ // 128      # 32 y j-tiles
YCH = 2              # y cast-DMA chunks
YCT = TYJ // YCH     # 16 j-tiles per chunk
ACW = 1024           # ACT's share of each [128, 2048] tile (bank aligned)

f32 = mybir.dt.float32
bf16 = mybir.dt.bfloat16
AF = mybir.ActivationFunctionType
ALU = mybir.AluOpType
AX = mybir.AxisListType

_CACHE = {}


def _build():
    if "nc" in _CACHE:
        return _CACHE["nc"]
    nc = bacc.Bacc("TRN2", target_bir_lowering=False, debug=False,
                   num_devices=NCORES)
    xs_d = nc.dram_tensor("xs", [XI, D], f32, kind="ExternalInput")
    y_d = nc.dram_tensor("y", [YJ, D], f32, kind="ExternalInput")
    yd_d = nc.dram_tensor("yd", [XI, D], f32, kind="ExternalInput")
    out_d = nc.dram_tensor("out", [128, 2], f32, kind="ExternalOutput")

    with tile.TileContext(nc) as tc:
        with (
            tc.tile_pool(name="singles", bufs=1) as singles,
            tc.tile_pool(name="scr", bufs=2) as scr,
        ):
            ident = singles.tile([128, 128], bf16)
            make_identity(nc, ident[:])
            onecol = singles.tile([128, 1], bf16)
            nc.vector.memset(onecol[:], 1.0)
            warm = singles.tile([128, 1], f32)
            nc.vector.memset(warm[:], 1.0)
            nc.scalar.sqrt(warm[:], warm[:])   # preload sqrt table set early

            ynat = singles.tile([128, TYJ, 128], bf16)
            yT = singles.tile([128, TYJ, 128], bf16)     # [d, t, j-col]
            xnat = singles.tile([128, TXI, 128], f32)    # row i=1024h+8p+tl
            xhat = singles.tile([128, TXI, 128], bf16)
            xhatT = singles.tile([128, TXI * 128], bf16)
            ydn = singles.tile([128, TXI, 128], bf16)
            ydT = singles.tile([128, TXI * 128], bf16)
            prodT = singles.tile([128, TXI * 128], bf16)

            ny2 = singles.tile([128, TYJ], f32)
            t2y = singles.tile([128, TYJ], f32)
            rny = singles.tile([128, TYJ], f32)
            nx2 = singles.tile([128, TXI], f32)
            t1x = singles.tile([128, TXI], f32)
            rnx = singles.tile([128, TXI], f32)
            nd64 = singles.tile([128, 64], f32)   # d2 (0:16) | nyd2 (16:32)
            rnyd = singles.tile([128, TXI], f32)
            sim_d = singles.tile([128, TXI], f32)
            relu_d = singles.tile([128, TXI], f32)
            R = singles.tile([128, 64], f32)
            outsb = singles.tile([128, 2], f32)

            # ---- input DMAs: x halves first (long pole), then y, yd
            for h in range(2):
                rows = slice(1024 * h, 1024 * (h + 1))
                nc.sync.dma_start(
                    out=xnat[:, 8 * h:8 * (h + 1), :],
                    in_=xs_d[rows].rearrange("(p t) d -> p t d", t=8))
            for g in range(YCH):
                rows = slice(2048 * g, 2048 * (g + 1))
                nc.gpsimd.dma_start(
                    out=ynat[:, YCT * g:YCT * (g + 1), :],
                    in_=y_d[rows].rearrange("(p t) d -> p t d", t=YCT))
            nc.gpsimd.dma_start(
                out=ydn[:, 0:8, :],
                in_=yd_d[0:1024].rearrange("(p t) d -> p t d", t=8))
            nc.gpsimd.dma_start(
                out=ydn[:, 8:16, :],
                in_=yd_d[1024:2048].rearrange("(p t) d -> p t d", t=8))

            # ---- x norms: DVE STT sumsq per tile (fp32), pipelined
            for t in range(TXI):
                nc.vector.scalar_tensor_tensor(
                    out=scr.tile([128, 128], f32, tag="st", name="st")[:],
                    in0=xnat[:, t, :], scalar=1.0, in1=xnat[:, t, :],
                    op0=ALU.mult, op1=ALU.mult,
                    accum_out=nx2[:, t:t + 1])
            nc.vector.reciprocal(t1x[:], nx2[:])
            nc.scalar.sqrt(rnx[:], t1x[:])   # 1/||x_i||
            for t in range(TXI):
                nc.vector.tensor_scalar(
                    out=xhat[:, t, :], in0=xnat[:, t, :],
                    scalar1=rnx[:, t:t + 1], scalar2=None,
                    op0=ALU.mult)

            # ---- y norms chunk 0: GpSimd squares (natural) + DVE reduce
            with nc.allow_low_precision("norm sums in bf16 are plenty"):
                ysq0 = scr.tile([128, YCT, 128], bf16, tag="sq", name="ysq0")
                nc.gpsimd.tensor_mul(
                    ysq0[:].rearrange("p a b -> p (a b)"),
                    ynat[:, 0:YCT, :].rearrange("p a b -> p (a b)"),
                    ynat[:, 0:YCT, :].rearrange("p a b -> p (a b)"))
                ny2h = scr.tile([128, YCT], bf16, tag="nyh", name="nyh")
                nc.vector.tensor_reduce(out=ny2h[:], in_=ysq0[:], axis=AX.X,
                                        op=ALU.add)
                nc.vector.tensor_copy(out=ny2[:, 0:YCT], in_=ny2h[:])

            # ---- transposes on PE (bf16) + copies to SBUF
            with tc.tile_pool(name="tpsum", bufs=2, space="PSUM") as tpsum:
                ptx = tpsum.tile([128, 2048], bf16, tag="tp")
                for t in range(TXI):
                    nc.tensor.transpose(ptx[:, 128 * t:128 * (t + 1)],
                                        xhat[:, t, :], ident[:])
                nc.scalar.copy(out=xhatT[:], in_=ptx[:])
                for g in range(YCH):
                    pty = tpsum.tile([128, 2048], bf16, tag="tp")
                    for k in range(YCT):
                        t = YCT * g + k
                        nc.tensor.transpose(pty[:, 128 * k:128 * (k + 1)],
                                            ynat[:, t, :], ident[:])
                    nc.vector.tensor_copy(
                        out=yT[:, YCT * g:YCT * (g + 1), :]
                        .rearrange("p a b -> p (a b)"),
                        in_=pty[:])
                ptd = tpsum.tile([128, 2048], bf16, tag="tp")
                for t in range(TXI):
                    nc.tensor.transpose(ptd[:, 128 * t:128 * (t + 1)],
                                        ydn[:, t, :], ident[:])
                nc.vector.tensor_copy(out=ydT[:], in_=ptd[:])

            # ---- y norms chunk 1 + diag sums via squares + ones-matmuls
            with nc.allow_low_precision("norm sums in bf16 are plenty"):
                ysqT = scr.tile([128, YCT * 128], bf16, tag="sq", name="ysqT")
                nc.gpsimd.tensor_mul(
                    ysqT[:],
                    yT[:, YCT:TYJ, :].rearrange("p a b -> p (a b)"),
                    yT[:, YCT:TYJ, :].rearrange("p a b -> p (a b)"))
                nc.vector.tensor_mul(prodT[:], xhatT[:], ydT[:])
                ydsqT = scr.tile([128, TXI * 128], bf16, tag="sq",
                                 name="ydsqT")
                nc.gpsimd.tensor_mul(ydsqT[:], ydT[:], ydT[:])

            with tc.tile_pool(name="npsum", bufs=1, space="PSUM") as npsum:
                pn = npsum.tile([128, 64], f32, tag="pn")
                for k in range(YCT):       # ny2 chunk1 -> cols 16+k? no: 0:16
                    nc.tensor.matmul(pn[:, k:k + 1],
                                     ysqT[:, 128 * k:128 * (k + 1)],
                                     onecol[:])
                for t in range(TXI):       # d2 -> cols 16:32
                    nc.tensor.matmul(pn[:, 16 + t:17 + t],
                                     prodT[:, 128 * t:128 * (t + 1)],
                                     onecol[:])
                for t in range(TXI):       # nyd2 -> cols 32:48
                    nc.tensor.matmul(pn[:, 32 + t:33 + t],
                                     ydsqT[:, 128 * t:128 * (t + 1)],
                                     onecol[:])
                nc.vector.tensor_copy(out=nd64[:, 0:48], in_=pn[:, 0:48])
            nc.vector.tensor_copy(out=ny2[:, YCT:TYJ], in_=nd64[:, 0:16])

            nc.vector.reciprocal(t2y[:], ny2[:])
            nc.scalar.sqrt(rny[:], t2y[:])   # 1/||y_j||

            # ---- main loop: 32 j-tiles of [128, 2048], both engines/tile
            with tc.tile_pool(name="mp", bufs=2, space="PSUM") as mp:
                for t in range(TYJ):
                    lhsT = yT[:, t, :]
                    ps = mp.tile([128, 2048], f32, tag="ps")
                    for k in range(4):
                        nc.tensor.matmul(ps[:, 512 * k:512 * (k + 1)],
                                         lhsT,
                                         xhatT[:, 512 * k:512 * (k + 1)])
                    nc.scalar.activation(
                        ps[:, 0:ACW], ps[:, 0:ACW], AF.Relu,
                        scale=rny[:, t:t + 1],
                        accum_out=R[:, 2 * t:2 * t + 1])
                    nc.vector.tensor_scalar(
                        out=ps[:, ACW:2048], in0=ps[:, ACW:2048],
                        scalar1=0.0, scalar2=None,
                        op0=ALU.max, op1=ALU.add,
                        accum_out=R[:, 2 * t + 1:2 * t + 2])

            # post-scale DVE R columns (odd) by rny
            nc.vector.tensor_mul(R[:, 1:64:2], R[:, 1:64:2], rny[:, 0:TYJ])

            # ---- diagonal scalars: sim_ii = d2 * rnyd  (d2 has rnx folded)
            with nc.allow_low_precision("diag in bf16 is plenty"):
                nc.vector.reciprocal(t1x[:], nd64[:, 32:48])
            nc.scalar.sqrt(rnyd[:], t1x[:])
            nc.vector.tensor_mul(sim_d[:], nd64[:, 16:32], rnyd[:])
            nc.scalar.activation(relu_d[:], sim_d[:], AF.Relu)
            nc.vector.scalar_tensor_tensor(
                out=scr.tile([128, TXI], f32, tag="dd", name="dd")[:],
                in0=sim_d[:], scalar=1.0, in1=relu_d[:],
                op0=ALU.mult, op1=ALU.add, accum_out=outsb[:, 1:2])

            # ---- final: sum R columns
            nc.vector.tensor_reduce(out=outsb[:, 0:1], in_=R[:],
                                    axis=AX.X, op=ALU.add)
            nc.sync.dma_start(out=out_d[:], in_=outsb[:])

    nc.compile()
    _CACHE["nc"] = nc
    return nc


# cores whose x block lies inside their y range own the diag correction
_DIAG_OWNER = [1, 0, 1, 0, 0, 1, 0, 1]


def _in_maps(x, y):
    maps = []
    for c in range(NCORES):
        bi, bj = c // 2, c % 2
        xsl = slice(XI * bi, XI * (bi + 1))
        ysl = slice(YJ * bj, YJ * (bj + 1))
        maps.append({"xs": np.ascontiguousarray(x[xsl]),
                     "y": np.ascontiguousarray(y[ysl]),
                     "yd": np.ascontiguousarray(y[xsl])})
    return maps


def _combine(results):
    total = 0.0
    for c in range(NCORES):
        o = results[c]["out"].astype(np.float64)
        total += o[:, 0].sum()
        if _DIAG_OWNER[c]:
            total += XI - o[:, 1].sum()
    return np.float32(total / (float(N) * float(N)))


def _run(x, y, trace=False):
    nc = _build()
    res = run_bass_kernel_spmd(nc, _in_maps(x, y), list(range(NCORES)),
                               trace=trace)
    return _combine(res.results), res


def kernel(x, y):
    x = np.asarray(x, dtype=np.float32)
    y = np.asarray(y, dtype=np.float32)
    loss, _ = _run(x, y, trace=False)
    return loss


# revision 8
# speedup vs baseline: 1.0854x; 1.0854x over previous
"""CrossCosineEmbeddingLoss kernel for 8 trn2 NeuronCores (v6).

loss = mean over all (i,j) of: 1 - cos(x_i, y_j) if i==j else relu(cos(x_i, y_j))

Identity:  total = sum_ij relu(sim_ij) + sum_i (1 - sim_ii - relu(sim_ii))

Sharding (2x4 grid): core c = (bi, bj), bi = c // 2, bj = c % 2.
  x rows [2048*bi, 2048*(bi+1)) x y rows [4096*bj, 4096*(bj+1)).
Each core computes sum_ij relu(x_hat_i . y_j) / ||y_j|| over its block.
Diag correction only used from cores whose x block lies in their y range.

Per-core pipeline:
  - x: 2 half-DMAs; DVE STT sumsq per tile; rsqrt; DVE scale-cast to bf16;
    PE transpose; ACT copy -> x_hatT  (pipelined at half granularity)
  - y: SWDGE cast-DMA fp32->bf16 (2 chunks); PE transpose + DVE copy -> yT.
    Norms chunk 0: GpSimd squares (natural) + DVE segmented reduce (fast
    path for rny). Chunk 1 + diag: GpSimd squares of transposed tiles +
    N=1 ones-matmuls on PE (column sums) + one small DVE copy.
  - main: 32 j-tiles, each [128j, 2048i] fp32 PSUM (4 banks, pool bufs=2):
    4 bf16 matmuls (N=512); tile split between engines on distinct banks:
      ACT: activation(Relu, scale=rny_col, accum_out) on cols [0:1024)
      DVE: tensor_scalar(max 0, add, accum_out) on cols [1024:2048),
           R column post-scaled by rny afterwards
Host combines [128,2] partials; diag col used only from owner cores.
"""

import numpy as np

import concourse.bacc as bacc
import concourse.bass as bass
import concourse.tile as tile
from concourse import mybir
from concourse.bass_utils import run_bass_kernel_spmd
from concourse.masks import make_identity

N, D = 8192, 128
NCORES = 8
XI = 2048            # x rows per core
YJ = 4096            # y rows per core
TXI = XI // 128      # 16 x tiles
TYJ = YJ // 128      # 32 y j-tiles
YCH = 2              # y cast-DMA chunks
YCT = TYJ // YCH     # 16 j-tiles per chunk
ACW = 1024           # ACT's share of each [128, 2048] tile (bank aligned)

f32 = mybir.dt.float32
bf16 = mybir.dt.bfloat16
AF = mybir.ActivationFunctionType
ALU = mybir.AluOpType
AX = mybir.AxisListType

_CACHE = {}


def _build():
    if "nc" in _CACHE:
        return _CACHE["nc"]
    nc = bacc.Bacc("TRN2", target_bir_lowering=False, debug=False,
                   num_devices=NCORES)
    xs_d = nc.dram_tensor("xs", [XI, D], f32, kind="ExternalInput")
    y_d = nc.dram_tensor("y", [YJ, D], f32, kind="ExternalInput")
    yd_d = nc.dram_tensor("yd", [XI, D], f32, kind="ExternalInput")
    out_d = nc.dram_tensor("out", [128, 2], f32, kind="ExternalOutput")

    with tile.TileContext(nc) as tc:
        with (
            tc.tile_pool(name="singles", bufs=1) as singles,
            tc.tile_pool(name="scr", bufs=2) as scr,
        ):
            ident = singles.tile([128, 128], bf16)
            make_identity(nc, ident[:])
            onecol = singles.tile([128, 1], bf16)
            nc.vector.memset(onecol[:], 1.0)
            warm = singles.tile([128, 1], f32)
            nc.vector.memset(warm[:], 1.0)
            nc.scalar.sqrt(warm[:], warm[:])   # preload sqrt table set early

            ynat = singles.tile([128, TYJ, 128], bf16)
            yT = singles.tile([128, TYJ, 128], bf16)     # [d, t, j-col]
            xnat = singles.tile([128, TXI, 128], f32)    # row i=1024h+8p+tl
            xhat = singles.tile([128, TXI, 128], bf16)
            xhatT = singles.tile([128, TXI * 128], bf16)
            ydn = singles.tile([128, TXI, 128], bf16)
            ydT = singles.tile([128, TXI * 128], bf16)
            prodT = singles.tile([128, TXI * 128], bf16)

            ny2 = singles.tile([128, TYJ], f32)
            t2y = singles.tile([128, TYJ], f32)
            rny = singles.tile([128, TYJ], f32)
            nx2 = singles.tile([128, TXI], f32)
            t1x = singles.tile([128, TXI], f32)
            rnx = singles.tile([128, TXI], f32)
            nd64 = singles.tile([128, 64], f32)   # d2 (0:16) | nyd2 (16:32)
            rnyd = singles.tile([128, TXI], f32)
            sim_d = singles.tile([128, TXI], f32)
            relu_d = singles.tile([128, TXI], f32)
            R = singles.tile([128, 64], f32)
            outsb = singles.tile([128, 2], f32)

            # ---- input DMAs: x halves first (long pole), then y, yd
            for h in range(2):
                rows = slice(1024 * h, 1024 * (h + 1))
                nc.sync.dma_start(
                    out=xnat[:, 8 * h:8 * (h + 1), :],
                    in_=xs_d[rows].rearrange("(p t) d -> p t d", t=8))
            for g in range(YCH):
                rows = slice(2048 * g, 2048 * (g + 1))
                nc.gpsimd.dma_start(
                    out=ynat[:, YCT * g:YCT * (g + 1), :],
                    in_=y_d[rows].rearrange("(p t) d -> p t d", t=YCT))
            nc.gpsimd.dma_start(
                out=ydn[:, 0:8, :],
                in_=yd_d[0:1024].rearrange("(p t) d -> p t d", t=8))
            nc.gpsimd.dma_start(
                out=ydn[:, 8:16, :],
                in_=yd_d[1024:2048].rearrange("(p t) d -> p t d", t=8))

            # ---- x norms: DVE STT sumsq per tile (fp32), pipelined
            for t in range(TXI):
                nc.vector.scalar_tensor_tensor(
                    out=scr.tile([128, 128], f32, tag="st", name="st")[:],
                    in0=xnat[:, t, :], scalar=1.0, in1=xnat[:, t, :],
                    op0=ALU.mult, op1=ALU.mult,
                    accum_out=nx2[:, t:t + 1])
            nc.vector.reciprocal(t1x[:], nx2[:])
            nc.scalar.sqrt(rnx[:], t1x[:])   # 1/||x_i||
            for t in range(TXI):
                nc.vector.tensor_scalar(
                    out=xhat[:, t, :], in0=xnat[:, t, :],
                    scalar1=rnx[:, t:t + 1], scalar2=None,
                    op0=ALU.mult)

            # ---- y norms chunk 0: GpSimd squares (natural) + DVE reduce
            with nc.allow_low_precision("norm sums in bf16 are plenty"):
                ysq0 = scr.tile([128, YCT, 128], bf16, tag="sq", name="ysq0")
                nc.gpsimd.tensor_mul(
                    ysq0[:].rearrange("p a b -> p (a b)"),
                    ynat[:, 0:YCT, :].rearrange("p a b -> p (a b)"),
                    ynat[:, 0:YCT, :].rearrange("p a b -> p (a b)"))
                ny2h = scr.tile([128, YCT], bf16, tag="nyh", name="nyh")
                nc.vector.tensor_reduce(out=ny2h[:], in_=ysq0[:], axis=AX.X,
                                        op=ALU.add)
                nc.vector.tensor_copy(out=ny2[:, 0:YCT], in_=ny2h[:])

            # ---- transposes on PE (bf16) + copies to SBUF
            with tc.tile_pool(name="tpsum", bufs=2, space="PSUM") as tpsum:
                ptx = tpsum.tile([128, 2048], bf16, tag="tp")
                for t in range(TXI):
                    nc.tensor.transpose(ptx[:, 128 * t:128 * (t + 1)],
                                        xhat[:, t, :], ident[:])
                nc.scalar.copy(out=xhatT[:], in_=ptx[:])
                for g in range(YCH):
                    pty = tpsum.tile([128, 2048], bf16, tag="tp")
                    for k in range(YCT):
                        t = YCT * g + k
                        nc.tensor.transpose(pty[:, 128 * k:128 * (k + 1)],
                                            ynat[:, t, :], ident[:])
                    nc.vector.tensor_copy(
                        out=yT[:, YCT * g:YCT * (g + 1), :]
                        .rearrange("p a b -> p (a b)"),
                        in_=pty[:])
                ptd = tpsum.tile([128, 2048], bf16, tag="tp")
                for t in range(TXI):
                    nc.tensor.transpose(ptd[:, 128 * t:128 * (t + 1)],
                                        ydn[:, t, :], ident[:])
                nc.vector.tensor_copy(out=ydT[:], in_=ptd[:])

            # ---- y norms chunk 1 + diag sums via squares + ones-matmuls
            with nc.allow_low_precision("norm sums in bf16 are plenty"):
                ysqT = scr.tile([128, YCT * 128], bf16, tag="sq", name="ysqT")
                nc.gpsimd.tensor_mul(
                    ysqT[:],
                    yT[:, YCT:TYJ, :].rearrange("p a b -> p (a b)"),
                    yT[:, YCT:TYJ, :].rearrange("p a b -> p (a b)"))
                nc.vector.tensor_mul(prodT[:], xhatT[:], ydT[:])
                ydsqT = scr.tile([128, TXI * 128], bf16, tag="sq",
                                 name="ydsqT")
                nc.gpsimd.tensor_mul(ydsqT[:], ydT[:], ydT[:])

            with tc.tile_pool(name="npsum", bufs=1, space="PSUM") as npsum:
                pn = npsum.tile([128, 64], f32, tag="pn")
                for k in range(YCT):       # ny2 chunk1 -> cols 16+k? no: 0:16
                    nc.tensor.matmul(pn[:, k:k + 1],
                                     ysqT[:, 128 * k:128 * (k + 1)],
                                     onecol[:])
                for t in range(TXI):       # d2 -> cols 16:32
                    nc.tensor.matmul(pn[:, 16 + t:17 + t],
                                     prodT[:, 128 * t:128 * (t + 1)],
                                     onecol[:])
                for t in range(TXI):       # nyd2 -> cols 32:48
                    nc.tensor.matmul(pn[:, 32 + t:33 + t],
                                     ydsqT[:, 128 * t:128 * (t + 1)],
                                     onecol[:])
                nc.vector.tensor_copy(out=nd64[:, 0:48], in_=pn[:, 0:48])
            nc.vector.tensor_copy(out=ny2[:, YCT:TYJ], in_=nd64[:, 0:16])

            nc.vector.reciprocal(t2y[:], ny2[:])
            nc.scalar.sqrt(rny[:], t2y[:])   # 1/||y_j||

            # ---- main loop: 32 j-tiles of [128, 2048]; per tile the two
            # halves go to separate PSUM pool tiles so the ACT and DVE
            # reduces run concurrently (same-tile ops get serialized).
            with (
                tc.tile_pool(name="mpa", bufs=2, space="PSUM") as mpa,
                tc.tile_pool(name="mpd", bufs=2, space="PSUM") as mpd,
            ):
                for t in range(TYJ):
                    lhsT = yT[:, t, :]
                    pa = mpa.tile([128, ACW], f32, tag="pa")
                    pd = mpd.tile([128, 2048 - ACW], f32, tag="pd")
                    for k in range(4):
                        col = 512 * k
                        dst = (pa[:, col:col + 512] if col < ACW
                               else pd[:, col - ACW:col - ACW + 512])
                        nc.tensor.matmul(dst, lhsT,
                                         xhatT[:, col:col + 512])
                    nc.scalar.activation(
                        pa[:], pa[:], AF.Relu,
                        scale=rny[:, t:t + 1],
                        accum_out=R[:, 2 * t:2 * t + 1])
                    nc.vector.tensor_scalar(
                        out=pd[:], in0=pd[:],
                        scalar1=0.0, scalar2=None,
                        op0=ALU.max, op1=ALU.add,
                        accum_out=R[:, 2 * t + 1:2 * t + 2])

            # post-scale DVE R columns (odd) by rny
            nc.vector.tensor_mul(R[:, 1:64:2], R[:, 1:64:2], rny[:, 0:TYJ])

            # ---- diagonal scalars: sim_ii = d2 * rnyd  (d2 has rnx folded)
            with nc.allow_low_precision("diag in bf16 is plenty"):
                nc.vector.reciprocal(t1x[:], nd64[:, 32:48])
            nc.scalar.sqrt(rnyd[:], t1x[:])
            nc.vector.tensor_mul(sim_d[:], nd64[:, 16:32], rnyd[:])
            nc.scalar.activation(relu_d[:], sim_d[:], AF.Relu)
            nc.vector.scalar_tensor_tensor(
                out=scr.tile([128, TXI], f32, tag="dd", name="dd")[:],
                in0=sim_d[:], scalar=1.0, in1=relu_d[:],
                op0=ALU.mult, op1=ALU.add, accum_out=outsb[:, 1:2])

            # ---- final: sum R columns
            nc.vector.tensor_reduce(out=outsb[:, 0:1], in_=R[:],
                                    axis=AX.X, op=ALU.add)
            nc.sync.dma_start(out=out_d[:], in_=outsb[:])

    nc.compile()
    _CACHE["nc"] = nc
    return nc


# cores whose x block lies inside their y range own the diag correction
_DIAG_OWNER = [1, 0, 1, 0, 0, 1, 0, 1]


def _in_maps(x, y):
    maps = []
    for c in range(NCORES):
        bi, bj = c // 2, c % 2
        xsl = slice(XI * bi, XI * (bi + 1))
        ysl = slice(YJ * bj, YJ * (bj + 1))
        maps.append({"xs": np.ascontiguousarray(x[xsl]),
                     "y": np.ascontiguousarray(y[ysl]),
                     "yd": np.ascontiguousarray(y[xsl])})
    return maps


def _combine(results):
    total = 0.0
    for c in range(NCORES):
        o = results[c]["out"].astype(np.float64)
        total += o[:, 0].sum()
        if _DIAG_OWNER[c]:
            total += XI - o[:, 1].sum()
    return np.float32(total / (float(N) * float(N)))


def _run(x, y, trace=False):
    nc = _build()
    res = run_bass_kernel_spmd(nc, _in_maps(x, y), list(range(NCORES)),
                               trace=trace)
    return _combine(res.results), res


def kernel(x, y):
    x = np.asarray(x, dtype=np.float32)
    y = np.asarray(y, dtype=np.float32)
    loss, _ = _run(x, y, trace=False)
    return loss


# revision 10
# speedup vs baseline: 1.2554x; 1.1566x over previous
"""CrossCosineEmbeddingLoss kernel for 8 trn2 NeuronCores (v6).

loss = mean over all (i,j) of: 1 - cos(x_i, y_j) if i==j else relu(cos(x_i, y_j))

Identity:  total = sum_ij relu(sim_ij) + sum_i (1 - sim_ii - relu(sim_ii))

Sharding (2x4 grid): core c = (bi, bj), bi = c // 2, bj = c % 2.
  x rows [2048*bi, 2048*(bi+1)) x y rows [4096*bj, 4096*(bj+1)).
Each core computes sum_ij relu(x_hat_i . y_j) / ||y_j|| over its block.
Diag correction only used from cores whose x block lies in their y range.

Per-core pipeline:
  - x: 2 half-DMAs; DVE STT sumsq per tile; rsqrt; DVE scale-cast to bf16;
    PE transpose; ACT copy -> x_hatT  (pipelined at half granularity)
  - y: SWDGE cast-DMA fp32->bf16 (2 chunks); PE transpose + DVE copy -> yT.
    Norms chunk 0: GpSimd squares (natural) + DVE segmented reduce (fast
    path for rny). Chunk 1 + diag: GpSimd squares of transposed tiles +
    N=1 ones-matmuls on PE (column sums) + one small DVE copy.
  - main: 32 j-tiles, each [128j, 2048i] fp32 PSUM (4 banks, pool bufs=2):
    4 bf16 matmuls (N=512); tile split between engines on distinct banks:
      ACT: activation(Relu, scale=rny_col, accum_out) on cols [0:1024)
      DVE: tensor_scalar(max 0, add, accum_out) on cols [1024:2048),
           R column post-scaled by rny afterwards
Host combines [128,2] partials; diag col used only from owner cores.
"""

import numpy as np

import concourse.bacc as bacc
import concourse.bass as bass
import concourse.tile as tile
from concourse import mybir
from concourse.bass_utils import run_bass_kernel_spmd
from concourse.masks import make_identity

N, D = 8192, 128
NCORES = 8
XI = 2048            # x rows per core
YJ = 4096            # y rows per core
TXI = XI // 128      # 16 x tiles
TYJ = YJ // 128      # 32 y j-tiles
YCH = 2              # y cast-DMA chunks
YCT = TYJ // YCH     # 16 j-tiles per chunk
ACW = 1024           # ACT's share of each [128, 2048] tile (bank aligned)

f32 = mybir.dt.float32
bf16 = mybir.dt.bfloat16
AF = mybir.ActivationFunctionType
ALU = mybir.AluOpType
AX = mybir.AxisListType

_CACHE = {}


def _build():
    if "nc" in _CACHE:
        return _CACHE["nc"]
    nc = bacc.Bacc("TRN2", target_bir_lowering=False, debug=False,
                   num_devices=NCORES)
    xs_d = nc.dram_tensor("xs", [XI, D], f32, kind="ExternalInput")
    y_d = nc.dram_tensor("y", [YJ, D], f32, kind="ExternalInput")
    yd_d = nc.dram_tensor("yd", [XI, D], f32, kind="ExternalInput")
    out_d = nc.dram_tensor("out", [128, 2], f32, kind="ExternalOutput")

    with tile.TileContext(nc) as tc:
        with (
            tc.tile_pool(name="singles", bufs=1) as singles,
            tc.tile_pool(name="scr", bufs=2) as scr,
        ):
            ident = singles.tile([128, 128], bf16)
            make_identity(nc, ident[:])
            onecol = singles.tile([128, 1], bf16)
            nc.vector.memset(onecol[:], 1.0)
            warm = singles.tile([128, 1], f32)
            nc.vector.memset(warm[:], 1.0)
            nc.scalar.sqrt(warm[:], warm[:])   # preload sqrt table set early

            ynat = singles.tile([128, TYJ, 128], bf16)
            yT = singles.tile([128, TYJ, 128], bf16)     # [d, t, j-col]
            xnat = singles.tile([128, TXI, 128], f32)    # row i=1024h+8p+tl
            xhat = singles.tile([128, TXI, 128], bf16)
            xhatT = singles.tile([128, TXI * 128], bf16)
            ydn = singles.tile([128, TXI, 128], bf16)
            ydT = singles.tile([128, TXI * 128], bf16)
            prodT = singles.tile([128, TXI * 128], bf16)

            ny2 = singles.tile([128, TYJ], f32)
            t2y = singles.tile([128, TYJ], f32)
            rny = singles.tile([128, TYJ], f32)
            nx2 = singles.tile([128, TXI], f32)
            t1x = singles.tile([128, TXI], f32)
            rnx = singles.tile([128, TXI], f32)
            nd64 = singles.tile([128, 64], f32)   # d2 (0:16) | nyd2 (16:32)
            rnyd = singles.tile([128, TXI], f32)
            sim_d = singles.tile([128, TXI], f32)
            relu_d = singles.tile([128, TXI], f32)
            R = singles.tile([128, 64], f32)
            outsb = singles.tile([128, 2], f32)

            # ---- input DMAs: x halves first (long pole), then y, yd
            for h in range(2):
                rows = slice(1024 * h, 1024 * (h + 1))
                nc.sync.dma_start(
                    out=xnat[:, 8 * h:8 * (h + 1), :],
                    in_=xs_d[rows].rearrange("(p t) d -> p t d", t=8))
            for g in range(YCH):
                rows = slice(2048 * g, 2048 * (g + 1))
                nc.gpsimd.dma_start(
                    out=ynat[:, YCT * g:YCT * (g + 1), :],
                    in_=y_d[rows].rearrange("(p t) d -> p t d", t=YCT))
            nc.gpsimd.dma_start(
                out=ydn[:, 0:8, :],
                in_=yd_d[0:1024].rearrange("(p t) d -> p t d", t=8))
            nc.gpsimd.dma_start(
                out=ydn[:, 8:16, :],
                in_=yd_d[1024:2048].rearrange("(p t) d -> p t d", t=8))

            # ---- x norms: DVE STT sumsq per tile (fp32), pipelined halves
            for h in range(2):
                hs = slice(8 * h, 8 * (h + 1))
                for t in range(8 * h, 8 * h + 8):
                    nc.vector.scalar_tensor_tensor(
                        out=scr.tile([128, 128], f32, tag="st", name="st")[:],
                        in0=xnat[:, t, :], scalar=1.0, in1=xnat[:, t, :],
                        op0=ALU.mult, op1=ALU.mult,
                        accum_out=nx2[:, t:t + 1])
                nc.vector.reciprocal(t1x[:, hs], nx2[:, hs])
                nc.scalar.sqrt(rnx[:, hs], t1x[:, hs])   # 1/||x_i||
                for t in range(8 * h, 8 * h + 8):
                    nc.vector.tensor_scalar(
                        out=xhat[:, t, :], in0=xnat[:, t, :],
                        scalar1=rnx[:, t:t + 1], scalar2=None,
                        op0=ALU.mult)

            # ---- y norms chunk 0: GpSimd squares (natural) + DVE reduce
            with nc.allow_low_precision("norm sums in bf16 are plenty"):
                ysq0 = scr.tile([128, YCT, 128], bf16, tag="sq", name="ysq0")
                nc.gpsimd.tensor_mul(
                    ysq0[:].rearrange("p a b -> p (a b)"),
                    ynat[:, 0:YCT, :].rearrange("p a b -> p (a b)"),
                    ynat[:, 0:YCT, :].rearrange("p a b -> p (a b)"))
                ny2h = scr.tile([128, YCT], bf16, tag="nyh", name="nyh")
                nc.vector.tensor_reduce(out=ny2h[:], in_=ysq0[:], axis=AX.X,
                                        op=ALU.add)
                nc.vector.tensor_copy(out=ny2[:, 0:YCT], in_=ny2h[:])

            # ---- transposes on PE (bf16) + copies to SBUF
            with tc.tile_pool(name="tpsum", bufs=2, space="PSUM") as tpsum:
                ptx = tpsum.tile([128, 2048], bf16, tag="tp")
                for t in range(TXI):
                    nc.tensor.transpose(ptx[:, 128 * t:128 * (t + 1)],
                                        xhat[:, t, :], ident[:])
                nc.scalar.copy(out=xhatT[:], in_=ptx[:])
                for g in range(YCH):
                    pty = tpsum.tile([128, 2048], bf16, tag="tp")
                    for k in range(YCT):
                        t = YCT * g + k
                        nc.tensor.transpose(pty[:, 128 * k:128 * (k + 1)],
                                            ynat[:, t, :], ident[:])
                    nc.vector.tensor_copy(
                        out=yT[:, YCT * g:YCT * (g + 1), :]
                        .rearrange("p a b -> p (a b)"),
                        in_=pty[:])
                ptd = tpsum.tile([128, 2048], bf16, tag="tp")
                for t in range(TXI):
                    nc.tensor.transpose(ptd[:, 128 * t:128 * (t + 1)],
                                        ydn[:, t, :], ident[:])
                nc.vector.tensor_copy(out=ydT[:], in_=ptd[:])

            # ---- y norms chunk 1 + diag sums via squares + ones-matmuls
            with nc.allow_low_precision("norm sums in bf16 are plenty"):
                ysqT = scr.tile([128, YCT * 128], bf16, tag="sq", name="ysqT")
                nc.gpsimd.tensor_mul(
                    ysqT[:],
                    yT[:, YCT:TYJ, :].rearrange("p a b -> p (a b)"),
                    yT[:, YCT:TYJ, :].rearrange("p a b -> p (a b)"))
                nc.vector.tensor_mul(prodT[:], xhatT[:], ydT[:])
                ydsqT = scr.tile([128, TXI * 128], bf16, tag="sq",
                                 name="ydsqT")
                nc.gpsimd.tensor_mul(ydsqT[:], ydT[:], ydT[:])

            with tc.tile_pool(name="npsum", bufs=1, space="PSUM") as npsum:
                pn = npsum.tile([128, 64], f32, tag="pn")
                for k in range(YCT):       # ny2 chunk1 -> cols 16+k? no: 0:16
                    nc.tensor.matmul(pn[:, k:k + 1],
                                     ysqT[:, 128 * k:128 * (k + 1)],
                                     onecol[:])
                for t in range(TXI):       # d2 -> cols 16:32
                    nc.tensor.matmul(pn[:, 16 + t:17 + t],
                                     prodT[:, 128 * t:128 * (t + 1)],
                                     onecol[:])
                for t in range(TXI):       # nyd2 -> cols 32:48
                    nc.tensor.matmul(pn[:, 32 + t:33 + t],
                                     ydsqT[:, 128 * t:128 * (t + 1)],
                                     onecol[:])
                nc.vector.tensor_copy(out=nd64[:, 0:48], in_=pn[:, 0:48])
            nc.vector.tensor_copy(out=ny2[:, YCT:TYJ], in_=nd64[:, 0:16])

            nc.vector.reciprocal(t2y[:], ny2[:])
            nc.scalar.sqrt(rny[:], t2y[:])   # 1/||y_j||

            # ---- main loop: 32 j-tiles of [128, 2048]; per tile the two
            # halves go to separate PSUM pool tiles so the ACT and DVE
            # reduces run concurrently (same-tile ops get serialized).
            with (
                tc.tile_pool(name="mpa", bufs=2, space="PSUM") as mpa,
                tc.tile_pool(name="mpd", bufs=2, space="PSUM") as mpd,
            ):
                dumpA = singles.tile([128, ACW], bf16)
                dumpD = singles.tile([128, 2048 - ACW], bf16)
                for t in range(TYJ):
                    lhsT = yT[:, t, :]
                    pa = mpa.tile([128, ACW], f32, tag="pa")
                    pd = mpd.tile([128, 2048 - ACW], f32, tag="pd")
                    for k in range(4):
                        col = 512 * k
                        dst = (pa[:, col:col + 512] if col < ACW
                               else pd[:, col - ACW:col - ACW + 512])
                        nc.tensor.matmul(dst, lhsT,
                                         xhatT[:, col:col + 512])
                    nc.scalar.activation(
                        dumpA[:], pa[:], AF.Relu,
                        scale=rny[:, t:t + 1],
                        accum_out=R[:, 2 * t:2 * t + 1])
                    nc.vector.tensor_scalar(
                        out=dumpD[:], in0=pd[:],
                        scalar1=0.0, scalar2=None,
                        op0=ALU.max, op1=ALU.add,
                        accum_out=R[:, 2 * t + 1:2 * t + 2])

            # post-scale DVE R columns (odd) by rny
            nc.vector.tensor_mul(R[:, 1:64:2], R[:, 1:64:2], rny[:, 0:TYJ])

            # ---- diagonal scalars: sim_ii = d2 * rnyd  (d2 has rnx folded)
            with nc.allow_low_precision("diag in bf16 is plenty"):
                nc.vector.reciprocal(t1x[:], nd64[:, 32:48])
            nc.scalar.sqrt(rnyd[:], t1x[:])
            nc.vector.tensor_mul(sim_d[:], nd64[:, 16:32], rnyd[:])
            nc.scalar.activation(relu_d[:], sim_d[:], AF.Relu)
            nc.vector.scalar_tensor_tensor(
                out=scr.tile([128, TXI], f32, tag="dd", name="dd")[:],
                in0=sim_d[:], scalar=1.0, in1=relu_d[:],
                op0=ALU.mult, op1=ALU.add, accum_out=outsb[:, 1:2])

            # ---- final: sum R columns
            nc.vector.tensor_reduce(out=outsb[:, 0:1], in_=R[:],
                                    axis=AX.X, op=ALU.add)
            nc.sync.dma_start(out=out_d[:], in_=outsb[:])

    nc.compile()
    _CACHE["nc"] = nc
    return nc


# cores whose x block lies inside their y range own the diag correction
_DIAG_OWNER = [1, 0, 1, 0, 0, 1, 0, 1]


def _in_maps(x, y):
    maps = []
    for c in range(NCORES):
        bi, bj = c // 2, c % 2
        xsl = slice(XI * bi, XI * (bi + 1))
        ysl = slice(YJ * bj, YJ * (bj + 1))
        maps.append({"xs": np.ascontiguousarray(x[xsl]),
                     "y": np.ascontiguousarray(y[ysl]),
                     "yd": np.ascontiguousarray(y[xsl])})
    return maps


def _combine(results):
    total = 0.0
    for c in range(NCORES):
        o = results[c]["out"].astype(np.float64)
        total += o[:, 0].sum()
        if _DIAG_OWNER[c]:
            total += XI - o[:, 1].sum()
    return np.float32(total / (float(N) * float(N)))


def _run(x, y, trace=False):
    nc = _build()
    res = run_bass_kernel_spmd(nc, _in_maps(x, y), list(range(NCORES)),
                               trace=trace)
    return _combine(res.results), res


def kernel(x, y):
    x = np.asarray(x, dtype=np.float32)
    y = np.asarray(y, dtype=np.float32)
    loss, _ = _run(x, y, trace=False)
    return loss


# revision 11
# speedup vs baseline: 1.3232x; 1.0540x over previous
"""CrossCosineEmbeddingLoss kernel for 8 trn2 NeuronCores (v6).

loss = mean over all (i,j) of: 1 - cos(x_i, y_j) if i==j else relu(cos(x_i, y_j))

Identity:  total = sum_ij relu(sim_ij) + sum_i (1 - sim_ii - relu(sim_ii))

Sharding (2x4 grid): core c = (bi, bj), bi = c // 2, bj = c % 2.
  x rows [2048*bi, 2048*(bi+1)) x y rows [4096*bj, 4096*(bj+1)).
Each core computes sum_ij relu(x_hat_i . y_j) / ||y_j|| over its block.
Diag correction only used from cores whose x block lies in their y range.

Per-core pipeline:
  - x: 2 half-DMAs; DVE STT sumsq per tile; rsqrt; DVE scale-cast to bf16;
    PE transpose; ACT copy -> x_hatT  (pipelined at half granularity)
  - y: SWDGE cast-DMA fp32->bf16 (2 chunks); PE transpose + DVE copy -> yT.
    Norms chunk 0: GpSimd squares (natural) + DVE segmented reduce (fast
    path for rny). Chunk 1 + diag: GpSimd squares of transposed tiles +
    N=1 ones-matmuls on PE (column sums) + one small DVE copy.
  - main: 32 j-tiles, each [128j, 2048i] fp32 PSUM (4 banks, pool bufs=2):
    4 bf16 matmuls (N=512); tile split between engines on distinct banks:
      ACT: activation(Relu, scale=rny_col, accum_out) on cols [0:1024)
      DVE: tensor_scalar(max 0, add, accum_out) on cols [1024:2048),
           R column post-scaled by rny afterwards
Host combines [128,2] partials; diag col used only from owner cores.
"""

import numpy as np

import concourse.bacc as bacc
import concourse.bass as bass
import concourse.tile as tile
from concourse import mybir
from concourse.bass_utils import run_bass_kernel_spmd
from concourse.masks import make_identity

N, D = 8192, 128
NCORES = 8
XI = 2048            # x rows per core
YJ = 4096            # y rows per core
TXI = XI // 128      # 16 x tiles
TYJ = YJ // 128      # 32 y j-tiles
YCH = 2              # y cast-DMA chunks
YCT = TYJ // YCH     # 16 j-tiles per chunk
ACW = 1024           # ACT's share of each [128, 2048] tile (bank aligned)

f32 = mybir.dt.float32
bf16 = mybir.dt.bfloat16
AF = mybir.ActivationFunctionType
ALU = mybir.AluOpType
AX = mybir.AxisListType

_CACHE = {}


def _build():
    if "nc" in _CACHE:
        return _CACHE["nc"]
    nc = bacc.Bacc("TRN2", target_bir_lowering=False, debug=False,
                   num_devices=NCORES)
    xs_d = nc.dram_tensor("xs", [XI, D], f32, kind="ExternalInput")
    y_d = nc.dram_tensor("y", [YJ, D], f32, kind="ExternalInput")
    yd_d = nc.dram_tensor("yd", [XI, D], f32, kind="ExternalInput")
    out_d = nc.dram_tensor("out", [128, 2], f32, kind="ExternalOutput")

    with tile.TileContext(nc) as tc:
        with (
            tc.tile_pool(name="singles", bufs=1) as singles,
            tc.tile_pool(name="scr", bufs=2) as scr,
        ):
            ident = singles.tile([128, 128], bf16)
            make_identity(nc, ident[:])
            onecol = singles.tile([128, 1], bf16)
            nc.vector.memset(onecol[:], 1.0)
            warm = singles.tile([128, 1], f32)
            nc.vector.memset(warm[:], 1.0)
            nc.scalar.sqrt(warm[:], warm[:])   # preload sqrt table set early

            ynat = singles.tile([128, TYJ, 128], bf16)
            yT = singles.tile([128, TYJ, 128], bf16)     # [d, t, j-col]
            xnat = singles.tile([128, TXI, 128], bf16)   # row i=1024h+8p+tl
            xhat = singles.tile([128, TXI, 128], bf16)
            xhatT = singles.tile([128, TXI * 128], bf16)
            ydn = singles.tile([128, TXI, 128], bf16)
            ydT = singles.tile([128, TXI * 128], bf16)
            prodT = singles.tile([128, TXI * 128], bf16)

            ny2 = singles.tile([128, TYJ], f32)
            t2y = singles.tile([128, TYJ], f32)
            rny = singles.tile([128, TYJ], f32)
            nx2 = singles.tile([128, TXI], f32)
            t1x = singles.tile([128, TXI], f32)
            rnx = singles.tile([128, TXI], f32)
            nd64 = singles.tile([128, 64], f32)   # d2 (0:16) | nyd2 (16:32)
            rnyd = singles.tile([128, TXI], f32)
            sim_d = singles.tile([128, TXI], f32)
            relu_d = singles.tile([128, TXI], f32)
            R = singles.tile([128, 64], f32)
            outsb = singles.tile([128, 2], f32)

            # ---- input DMAs: x halves first (long pole), then y, yd
            for h in range(2):
                rows = slice(1024 * h, 1024 * (h + 1))
                nc.gpsimd.dma_start(
                    out=xnat[:, 8 * h:8 * (h + 1), :],
                    in_=xs_d[rows].rearrange("(p t) d -> p t d", t=8))
            for g in range(YCH):
                rows = slice(2048 * g, 2048 * (g + 1))
                nc.gpsimd.dma_start(
                    out=ynat[:, YCT * g:YCT * (g + 1), :],
                    in_=y_d[rows].rearrange("(p t) d -> p t d", t=YCT))
            nc.gpsimd.dma_start(
                out=ydn[:, 0:8, :],
                in_=yd_d[0:1024].rearrange("(p t) d -> p t d", t=8))
            nc.gpsimd.dma_start(
                out=ydn[:, 8:16, :],
                in_=yd_d[1024:2048].rearrange("(p t) d -> p t d", t=8))

            # ---- x norms: DVE STT sumsq per tile (fp32), pipelined halves
            for h in range(2):
                hs = slice(8 * h, 8 * (h + 1))
                for t in range(8 * h, 8 * h + 8):
                    nc.vector.scalar_tensor_tensor(
                        out=scr.tile([128, 128], bf16, tag="st", name="st")[:],
                        in0=xnat[:, t, :], scalar=1.0, in1=xnat[:, t, :],
                        op0=ALU.mult, op1=ALU.mult,
                        accum_out=nx2[:, t:t + 1])
                nc.vector.reciprocal(t1x[:, hs], nx2[:, hs])
                nc.scalar.sqrt(rnx[:, hs], t1x[:, hs])   # 1/||x_i||
                for t in range(8 * h, 8 * h + 8):
                    nc.vector.tensor_scalar(
                        out=xhat[:, t, :], in0=xnat[:, t, :],
                        scalar1=rnx[:, t:t + 1], scalar2=None,
                        op0=ALU.mult)

            # ---- y norms chunk 0: GpSimd squares (natural) + DVE reduce
            with nc.allow_low_precision("norm sums in bf16 are plenty"):
                ysq0 = scr.tile([128, YCT, 128], bf16, tag="sq", name="ysq0")
                nc.gpsimd.tensor_mul(
                    ysq0[:].rearrange("p a b -> p (a b)"),
                    ynat[:, 0:YCT, :].rearrange("p a b -> p (a b)"),
                    ynat[:, 0:YCT, :].rearrange("p a b -> p (a b)"))
                ny2h = scr.tile([128, YCT], bf16, tag="nyh", name="nyh")
                nc.vector.tensor_reduce(out=ny2h[:], in_=ysq0[:], axis=AX.X,
                                        op=ALU.add)
                nc.vector.tensor_copy(out=ny2[:, 0:YCT], in_=ny2h[:])
            nc.vector.reciprocal(t2y[:, 0:YCT], ny2[:, 0:YCT])
            nc.scalar.sqrt(rny[:, 0:YCT], t2y[:, 0:YCT])

            # ---- transposes on PE (bf16) + copies to SBUF
            with tc.tile_pool(name="tpsum", bufs=2, space="PSUM") as tpsum:
                for h in range(2):
                    ptx = tpsum.tile([128, 1024], bf16, tag="tph")
                    for k in range(8):
                        t = 8 * h + k
                        nc.tensor.transpose(ptx[:, 128 * k:128 * (k + 1)],
                                            xhat[:, t, :], ident[:])
                    nc.scalar.copy(out=xhatT[:, 1024 * h:1024 * (h + 1)],
                                   in_=ptx[:])
                for g in range(YCH):
                    pty = tpsum.tile([128, 2048], bf16, tag="tp")
                    for k in range(YCT):
                        t = YCT * g + k
                        nc.tensor.transpose(pty[:, 128 * k:128 * (k + 1)],
                                            ynat[:, t, :], ident[:])
                    nc.vector.tensor_copy(
                        out=yT[:, YCT * g:YCT * (g + 1), :]
                        .rearrange("p a b -> p (a b)"),
                        in_=pty[:])
                ptd = tpsum.tile([128, 2048], bf16, tag="tp")
                for t in range(TXI):
                    nc.tensor.transpose(ptd[:, 128 * t:128 * (t + 1)],
                                        ydn[:, t, :], ident[:])
                nc.vector.tensor_copy(out=ydT[:], in_=ptd[:])

            # ---- y norms chunk 1 + diag sums via squares + ones-matmuls
            with nc.allow_low_precision("norm sums in bf16 are plenty"):
                ysqT = scr.tile([128, YCT * 128], bf16, tag="sq", name="ysqT")
                nc.gpsimd.tensor_mul(
                    ysqT[:],
                    yT[:, YCT:TYJ, :].rearrange("p a b -> p (a b)"),
                    yT[:, YCT:TYJ, :].rearrange("p a b -> p (a b)"))
                nc.vector.tensor_mul(prodT[:], xhatT[:], ydT[:])
                ydsqT = scr.tile([128, TXI * 128], bf16, tag="sq",
                                 name="ydsqT")
                nc.gpsimd.tensor_mul(ydsqT[:], ydT[:], ydT[:])

            with tc.tile_pool(name="npsum", bufs=1, space="PSUM") as npsum:
                pn = npsum.tile([128, 64], f32, tag="pn")
                for k in range(YCT):       # ny2 chunk1 -> cols 16+k? no: 0:16
                    nc.tensor.matmul(pn[:, k:k + 1],
                                     ysqT[:, 128 * k:128 * (k + 1)],
                                     onecol[:])
                for t in range(TXI):       # d2 -> cols 16:32
                    nc.tensor.matmul(pn[:, 16 + t:17 + t],
                                     prodT[:, 128 * t:128 * (t + 1)],
                                     onecol[:])
                for t in range(TXI):       # nyd2 -> cols 32:48
                    nc.tensor.matmul(pn[:, 32 + t:33 + t],
                                     ydsqT[:, 128 * t:128 * (t + 1)],
                                     onecol[:])
                nc.vector.tensor_copy(out=nd64[:, 0:48], in_=pn[:, 0:48])
            nc.vector.tensor_copy(out=ny2[:, YCT:TYJ], in_=nd64[:, 0:16])
            nc.vector.reciprocal(t2y[:, YCT:TYJ], ny2[:, YCT:TYJ])
            nc.scalar.sqrt(rny[:, YCT:TYJ], t2y[:, YCT:TYJ])

            # ---- main loop: 32 j-tiles of [128, 2048]; per tile the two
            # halves go to separate PSUM pool tiles so the ACT and DVE
            # reduces run concurrently (same-tile ops get serialized).
            with (
                tc.tile_pool(name="mpa", bufs=2, space="PSUM") as mpa,
                tc.tile_pool(name="mpd", bufs=2, space="PSUM") as mpd,
            ):
                dumpA = singles.tile([128, ACW], bf16)
                dumpD = singles.tile([128, 2048 - ACW], bf16)
                for t in range(TYJ):
                    lhsT = yT[:, t, :]
                    pa = mpa.tile([128, ACW], f32, tag="pa")
                    pd = mpd.tile([128, 2048 - ACW], f32, tag="pd")
                    for k in range(4):
                        col = 512 * k
                        dst = (pa[:, col:col + 512] if col < ACW
                               else pd[:, col - ACW:col - ACW + 512])
                        nc.tensor.matmul(dst, lhsT,
                                         xhatT[:, col:col + 512])
                    nc.scalar.activation(
                        dumpA[:], pa[:], AF.Relu,
                        scale=rny[:, t:t + 1],
                        accum_out=R[:, 2 * t:2 * t + 1])
                    nc.vector.tensor_scalar(
                        out=dumpD[:], in0=pd[:],
                        scalar1=0.0, scalar2=None,
                        op0=ALU.max, op1=ALU.add,
                        accum_out=R[:, 2 * t + 1:2 * t + 2])

            # post-scale DVE R columns (odd) by rny
            nc.vector.tensor_mul(R[:, 1:64:2], R[:, 1:64:2], rny[:, 0:TYJ])

            # ---- diagonal scalars: sim_ii = d2 * rnyd  (d2 has rnx folded)
            with nc.allow_low_precision("diag in bf16 is plenty"):
                nc.vector.reciprocal(t1x[:], nd64[:, 32:48])
            nc.scalar.sqrt(rnyd[:], t1x[:])
            nc.vector.tensor_mul(sim_d[:], nd64[:, 16:32], rnyd[:])
            nc.scalar.activation(relu_d[:], sim_d[:], AF.Relu)
            nc.vector.scalar_tensor_tensor(
                out=scr.tile([128, TXI], f32, tag="dd", name="dd")[:],
                in0=sim_d[:], scalar=1.0, in1=relu_d[:],
                op0=ALU.mult, op1=ALU.add, accum_out=outsb[:, 1:2])

            # ---- final: sum R columns
            nc.vector.tensor_reduce(out=outsb[:, 0:1], in_=R[:],
                                    axis=AX.X, op=ALU.add)
            nc.sync.dma_start(out=out_d[:], in_=outsb[:])

    nc.compile()
    _CACHE["nc"] = nc
    return nc


# cores whose x block lies inside their y range own the diag correction
_DIAG_OWNER = [1, 0, 1, 0, 0, 1, 0, 1]


def _in_maps(x, y):
    maps = []
    for c in range(NCORES):
        bi, bj = c // 2, c % 2
        xsl = slice(XI * bi, XI * (bi + 1))
        ysl = slice(YJ * bj, YJ * (bj + 1))
        maps.append({"xs": np.ascontiguousarray(x[xsl]),
                     "y": np.ascontiguousarray(y[ysl]),
                     "yd": np.ascontiguousarray(y[xsl])})
    return maps


def _combine(results):
    total = 0.0
    for c in range(NCORES):
        o = results[c]["out"].astype(np.float64)
        total += o[:, 0].sum()
        if _DIAG_OWNER[c]:
            total += XI - o[:, 1].sum()
    return np.float32(total / (float(N) * float(N)))


def _run(x, y, trace=False):
    nc = _build()
    res = run_bass_kernel_spmd(nc, _in_maps(x, y), list(range(NCORES)),
                               trace=trace)
    return _combine(res.results), res


def kernel(x, y):
    x = np.asarray(x, dtype=np.float32)
    y = np.asarray(y, dtype=np.float32)
    loss, _ = _run(x, y, trace=False)
    return loss
